# revision 5
# baseline (speedup 1.0000x reference)
"""Trainium2 Bass kernel for the contrastive loss problem.

Math (per batch element b, one NeuronCore each):
  feat (C=64, N=4000), prob (N,);  normal = prob < 0.5
  featn = l2-normalize(feat, axis=C);  s = (featn.T @ featn) / 0.1
  pos_loss = -log(mean_{m!=n, both normal} exp(s_mn) + 1e-6)
  neg_loss = mean_{m normal, n anomaly} -log(1 - sigmoid(s_mn) + 1e-6)
  result   = sum_b valid_b * (pos+neg) / max(#valid, 1)

Strategy (data-parallel over batch, 8 cores):
- pos phase: bf16 Gram upper blocks, ScalarE Exp + fused accumulate.
- neg phase: softplus(s) = s + ln(1+e^{-s}).  Sum s is linear -> exact on
  host.  The 1+e^{-s} factors are built by two engine paths feeding one
  product-fold stream:
    * ACT path (9/16 row blocks): Exp activation with scale (exact e^{-s});
    * DVE path (7/16): Schraudolph bit-trick exp - the PSUM value y =
      -(A/16) s is mapped to int16 bits (y*16 + BVAL, truncating convert on
      the DVE) which reinterpret as the bf16 value ~ e^{-s}.  BVAL is bias-
      centered so the sawtooth error cancels in expectation (validated to
      ~1e-4 of neg_sum on this input distribution).
  The neg Gram itself runs in fp8-e4m3 DoubleRow matmuls (PE does K=64 in
  one 0.5-cycle/col pass); fp8 noise on s is symmetric and sigmoid-damped.
  Folds: (1+e) via DVE tensor_scalar (4x mode), pair-products via DVE
  tensor_tensor (2x) and Pool tensor_tensor, Ln+accumulate every 8 units.
Masked (zero-padded) entries contribute exactly-known constants per path;
the host subtracts them in closed form.  Diagonal 128x128 blocks, normal
points beyond 2048 and anomaly points beyond na - na%8 are handled on the
host in f64 (cheap boundary work).
"""

import numpy as np

RW = 2048          # padded region width = 16 blocks of 128
NBLK = RW // 128   # 16 row blocks
UNIT = 2048        # PSUM staging tile width (4 banks); ping-pong 2 tiles
N_CORES = 8
EPS = 1e-6
_SQ10 = float(np.sqrt(10.0))
_A = 128.0 * 1.4426950408889634   # 128*log2(e): bf16-bits units per ln-unit
_BSHIFT = -7.3
_BVAL = 127.0 * 128.0 + _BSHIFT + 0.5   # +0.5: truncating convert -> round
_NEG_SCALE = -_A / 16.0                 # host pre-scale on rn (fp8 range)

# neg unit path assignment: True -> ACT exact-exp, False -> DVE Schraudolph
_PATH_A = [True, False, True, False, True, False, True, False,
           True, False, True, False, True, False, True, True]  # 9 A / 7 B


def _make_stream(block_col_ranges):
    segs, pos = [], 0
    for j, cs, ce in block_col_ranges:
        c = cs
        while c < ce:
            take = min(512 - (pos % 512), ce - c)
            segs.append((j, c, c + take, pos))
            pos += take
            c += take
    return segs, pos


_POS_SEGS, _POS_LEN = _make_stream([(j, 128 * j, RW) for j in range(NBLK)])
_NU_P = (_POS_LEN + UNIT - 1) // UNIT   # 9 units (last 1024 wide)
_N_GRP = (NBLK + 7) // 8                # 2 Ln groups of 8 neg units

_compiled = None


def _build():
    import concourse.bass as bass
    import concourse.mybir as mybir
    import concourse.tile as tile
    from concourse import bacc
    from concourse.hw_specs import get_activation_tables

    # Keep Exp and Ln in one activation table set (avoids table reloads).
    def _tables_pref(arch):
        t = get_activation_tables(arch)
        pref = "natural_log_exp_and_others"
        AFt = mybir.ActivationFunctionType
        return {k: (v if k == pref else v - {AFt.Exp, AFt.Ln})
                for k, v in t.items()}

    bacc.get_activation_tables = _tables_pref

    f32 = mybir.dt.float32
    bf16 = mybir.dt.bfloat16
    fp8 = mybir.dt.float8e4
    i16 = mybir.dt.int16
    AF = mybir.ActivationFunctionType
    ALU = mybir.AluOpType

    nc = bacc.Bacc("TRN2", target_bir_lowering=False, debug=False,
                   num_devices=N_CORES)
    rp_d = nc.dram_tensor("rp", [64, RW], bf16, kind="ExternalInput")
    rp8_d = nc.dram_tensor("rp8", [32, 2, RW], fp8, kind="ExternalInput")
    rn8_d = nc.dram_tensor("rn8", [32, 2, RW], fp8, kind="ExternalInput")
    accp_d = nc.dram_tensor("accp", [128, _NU_P], f32, kind="ExternalOutput")
    accn_d = nc.dram_tensor("accn", [128, _N_GRP], f32, kind="ExternalOutput")

    with tile.TileContext(nc) as tc:
        with (
            tc.tile_pool(name="sb", bufs=1) as sb,
            tc.tile_pool(name="scratch", bufs=2) as scratch_pool,
            tc.tile_pool(name="psum", bufs=2, space=bass.MemorySpace.PSUM) as pp,
        ):
            rp_sb = sb.tile([64, RW], bf16, tag="rp")
            rp8_sb = sb.tile([32, 2, RW], fp8, tag="rp8")
            rn8_sb = sb.tile([32, 2, RW], fp8, tag="rn8")
            nc.sync.dma_start(out=rp_sb[:], in_=rp_d.ap())
            nc.gpsimd.dma_start(out=rp8_sb[:], in_=rp8_d.ap())
            nc.gpsimd.dma_start(out=rn8_sb[:], in_=rn8_d.ap())

            acc_p = sb.tile([128, _NU_P], f32, tag="accp")
            acc_n = sb.tile([128, _N_GRP], f32, tag="accn")

            def emit_pos_unit(u):
                base = u * UNIT
                w = min(UNIT, _POS_LEN - base)
                ptile = pp.tile([128, UNIT], f32, tag="unit")
                for (j, c0, c1, pos) in _POS_SEGS:
                    if base <= pos < base + w:
                        nc.tensor.matmul(
                            ptile[:, pos - base:pos - base + (c1 - c0)],
                            rp_sb[:, j * 128:(j + 1) * 128],
                            rp_sb[:, c0:c1],
                            start=True, stop=True,
                        )
                st = scratch_pool.tile([128, UNIT], bf16, tag="pscr")
                nc.scalar.activation(st[:, :w], ptile[:, :w], AF.Exp,
                                     accum_out=acc_p[:, u:u + 1])

            # neg fold stream state
            state = {"lt": None, "fill": 0, "grp": 0}

            def emit_neg_unit(j):
                ptile = pp.tile([128, UNIT], f32, tag="unit")
                for c in range(0, UNIT, 512):
                    nc.tensor.matmul(
                        ptile[:, c:c + 512],
                        rp8_sb[:, :, j * 128:(j + 1) * 128],
                        rn8_sb[:, :, c:c + 512],
                        start=True, stop=True,
                        perf_mode=mybir.MatmulPerfMode.DoubleRow,
                    )
                ft = scratch_pool.tile([128, UNIT], bf16, tag="ft")
                if _PATH_A[j]:
                    et = scratch_pool.tile([128, UNIT], bf16, tag="et")
                    nc.scalar.activation(et[:], ptile[:], AF.Exp,
                                         scale=float(16.0 / _A))
                    nc.vector.tensor_scalar_add(ft[:], et[:], 1.0)
                else:
                    bits = scratch_pool.tile([128, UNIT], i16, tag="bits")
                    nc.vector.tensor_scalar(
                        out=bits[:], in0=ptile[:],
                        scalar1=16.0, scalar2=float(_BVAL),
                        op0=ALU.mult, op1=ALU.add)
                    nc.vector.tensor_scalar_add(ft[:], bits[:].bitcast(bf16),
                                                1.0)
                f1 = scratch_pool.tile([128, UNIT // 2], bf16, tag="f1")
                nc.vector.tensor_tensor(f1[:], ft[:, :UNIT // 2],
                                        ft[:, UNIT // 2:], op=ALU.mult)
                f2 = scratch_pool.tile([128, UNIT // 4], bf16, tag="f2")
                nc.gpsimd.tensor_tensor(f2[:], f1[:, :UNIT // 4],
                                        f1[:, UNIT // 4:], op=ALU.mult)
                if state["lt"] is None:
                    state["lt"] = scratch_pool.tile([128, UNIT], bf16,
                                                    tag="lt", name="lt")
                    state["fill"] = 0
                lt, fill = state["lt"], state["fill"]
                nc.gpsimd.tensor_tensor(lt[:, fill:fill + UNIT // 8],
                                        f2[:, :UNIT // 8], f2[:, UNIT // 8:],
                                        op=ALU.mult)
                state["fill"] += UNIT // 8
                if state["fill"] == UNIT:
                    g = state["grp"]
                    ld = scratch_pool.tile([128, UNIT], bf16, tag="ld")
                    nc.scalar.activation(ld[:], lt[:], AF.Ln,
                                         accum_out=acc_n[:, g:g + 1])
                    state["grp"] += 1
                    state["lt"] = None

            # interleave pos and neg units so ACT (pos, path-A) and
            # DVE (path-B converts, folds) stay co-busy
            pos_left = list(range(_NU_P))
            neg_left = list(range(NBLK))
            total = _NU_P + NBLK
            pos_credit = 0.0
            for i in range(total):
                pos_credit += _NU_P / total
                if pos_left and (pos_credit >= 1.0 or not neg_left):
                    pos_credit -= 1.0
                    emit_pos_unit(pos_left.pop(0))
                else:
                    emit_neg_unit(neg_left.pop(0))

            nc.sync.dma_start(out=accp_d.ap(), in_=acc_p[:])
            nc.sync.dma_start(out=accn_d.ap(), in_=acc_n[:])

    nc.compile()
    return nc


def _get_compiled():
    global _compiled
    if _compiled is None:
        _compiled = _build()
    return _compiled


def _chain8(f):
    """Device fold-tree value of 8 equal bf16 factors: ((f^2)^2)^2 in bf16."""
    import ml_dtypes
    BF = ml_dtypes.bfloat16
    x = np.float64(f)
    for _ in range(3):
        x = np.float64(BF(x * x))
    return float(x)


def _prepare(features, anomaly_prob):
    import ml_dtypes
    BF = ml_dtypes.bfloat16
    FP8 = ml_dtypes.float8_e4m3
    feat_all = np.asarray(features, dtype=np.float32)[..., 0]
    prob_all = np.asarray(anomaly_prob, dtype=np.float32)[:, 0, :, 0]
    BS, C, N = feat_all.shape
    in_maps, metas = [], []
    for b in range(BS):
        feat, prob = feat_all[b], prob_all[b]
        normal = prob < np.float32(0.5)
        nn = int(normal.sum())
        na = N - nn
        if na > RW or nn - RW > 512:
            return None, None
        norms = np.sqrt(np.sum(feat * feat, axis=0, dtype=np.float32))
        sc = (np.float32(_SQ10) /
              np.maximum(norms, np.float32(1e-12))).astype(np.float32)
        featsc = feat * sc[None, :]
        nd = min(nn, RW)
        na_dev = na - (na % 8)
        fn_all = featsc[:, normal]
        fa_all = featsc[:, ~normal]
        rp = np.zeros((C, RW), np.float32)
        rp[:, :nd] = fn_all[:, :nd]
        rn = np.zeros((C, RW), np.float32)
        rn[:, :na_dev] = fa_all[:, :na_dev]
        rp16 = rp.astype(BF)
        rp64 = rp16.astype(np.float64)
        rn16 = fa_all.astype(BF).astype(np.float64)   # (64, na) for host math
        ov64 = fn_all[:, nd:nn].astype(BF).astype(np.float64)

        # fp8 operands for the neg phase (DoubleRow k-split 0:32 / 32:64)
        rp8 = rp16.astype(np.float32).astype(FP8)
        rn8 = (rn * np.float32(_NEG_SCALE)).astype(FP8)
        rp8_dr = np.ascontiguousarray(
            rp8.reshape(2, 32, RW).transpose(1, 0, 2))
        rn8_dr = np.ascontiguousarray(
            rn8.reshape(2, 32, RW).transpose(1, 0, 2))

        # host diag-block sums (pos): f64 on bf16 operands
        D_full = 0.0
        S2 = 0.0
        for j in range(NBLK):
            r0, r1 = 128 * j, min(128 * (j + 1), nd)
            if r0 >= r1:
                break
            blk = rp64[:, r0:r1]
            e = np.exp(blk.T @ blk)
            D_full += float(e.sum())
            S2 += float(np.trace(e))

        # pos overflow normals (beyond RW)
        pos_extra = 0.0
        if nn > nd:
            s_cross = ov64.T @ rp64[:, :nd]
            pos_extra += 2.0 * float(np.exp(s_cross).sum())
            e_oo = np.exp(ov64.T @ ov64)
            pos_extra += float(e_oo.sum()) - float(np.trace(e_oo))

        # neg host extras: overflow normals x ALL anomalies, plus device
        # normals x remainder anomalies (na_dev..na)
        neg_extra = 0.0
        if nn > nd:
            s_on = ov64.T @ rn16[:, :na]
            sig = 1.0 / (1.0 + np.exp(-s_on))
            neg_extra += float(-np.log(1.0 - sig + EPS).sum())
        if na_dev < na:
            s_rem = rp64[:, :nd].T @ rn16[:, na_dev:na]
            sig = 1.0 / (1.0 + np.exp(-s_rem))
            neg_extra += float(-np.log(1.0 - sig + EPS).sum())

        # sum of device s over the whole neg tile (masked entries are 0):
        # y = rp8.T @ rn8_scaled ; s = -(16/A) * y
        rp8_64 = rp8.astype(np.float64)
        rn8_64 = rn8.astype(np.float64)
        sum_y = float(rp8_64.sum(axis=1) @ rn8_64.sum(axis=1))
        sum_s = (-16.0 / _A) * sum_y

        # masked fold-column corrections per unit path
        bits0 = np.int16(int(_BVAL))  # y=0 -> trunc(BVAL)
        eps0 = float(np.array([bits0], np.int16).astype(np.uint16)
                     .view(BF).astype(np.float64)[0])
        fA = float(BF(2.0))
        fB = float(BF(1.0 + eps0))
        lnC_A = float(np.log(np.float64(_chain8(fA))))
        lnC_B = float(np.log(np.float64(_chain8(fB))))
        corr = 0.0
        for j in range(NBLK):
            rows_real = min(max(nd - 128 * j, 0), 128)
            masked = 128 * (UNIT // 8) - rows_real * (na_dev // 8)
            corr += masked * (lnC_A if _PATH_A[j] else lnC_B)

        metas.append((nn, na, nd, na_dev, D_full, S2, pos_extra,
                      neg_extra, sum_s, corr))
        in_maps.append({"rp": rp16, "rp8": rp8_dr, "rn8": rn8_dr})
    return in_maps, metas


def _combine(results, metas):
    per_batch, n_valid = [], 0
    for r, (nn, na, nd, na_dev, D_full, S2, pos_extra,
            neg_extra, sum_s, corr) in zip(results, metas):
        TC = float(np.asarray(r["accp"], dtype=np.float64).sum())
        TN = float(np.asarray(r["accn"], dtype=np.float64).sum())
        fake_c = 0
        for j in range(NBLK):
            rows = min(max(nd - 128 * j, 0), 128)
            cols = max(nd - 128 * j, 0)
            fake_c += 128 * (RW - 128 * j) - rows * cols
        TC_real = TC - float(fake_c)
        pos_sum = 2.0 * TC_real - D_full - S2 + pos_extra
        pos_loss = -np.log(pos_sum / max(nn * (nn - 1), 1) + EPS)
        # device neg: sum over real pairs of softplus(-s) = softplus(s) - s
        neg_sum = (TN - corr) + sum_s + neg_extra
        neg_loss = neg_sum / max(nn * na, 1)
        if nn >= 10 and na >= 5:
            n_valid += 1
            per_batch.append(pos_loss + neg_loss)
    total = np.sum(per_batch) / max(n_valid, 1) if per_batch else 0.0
    return np.asarray(total, dtype=np.float32)


def _numpy_fallback(features, anomaly_prob):
    feat_all = np.asarray(features, dtype=np.float32)[..., 0]
    prob_all = np.asarray(anomaly_prob, dtype=np.float32)[:, 0, :, 0]
    BS, C, N = feat_all.shape
    per_batch, n_valid = [], 0
    for b in range(BS):
        feat, prob = feat_all[b], prob_all[b]
        normal = prob < 0.5
        nn = int(normal.sum()); na = N - nn
        norms = np.sqrt(np.sum(feat * feat, axis=0, dtype=np.float32))
        fn = feat / np.maximum(norms, 1e-12)[None, :]
        s = (fn.T @ fn) / np.float32(0.1)
        nm, am = normal, ~normal
        eye = np.eye(N, dtype=bool)
        pm = nm[:, None] & nm[None, :] & ~eye
        pos_mean = np.where(pm, np.exp(s), 0.0).sum() / max(pm.sum(), 1)
        pos_loss = -np.log(pos_mean + EPS)
        cm = nm[:, None] & am[None, :]
        neg = np.where(cm, -np.log(1.0 - 1.0 / (1.0 + np.exp(-s)) + EPS),
                       0.0).sum() / max(cm.sum(), 1)
        if nn >= 10 and na >= 5:
            n_valid += 1
            per_batch.append(pos_loss + neg)
    total = np.sum(per_batch) / max(n_valid, 1) if per_batch else 0.0
    return np.asarray(total, dtype=np.float32)


def kernel(features, anomaly_prob):
    from concourse.bass_utils import run_bass_kernel_spmd
    in_maps, metas = _prepare(features, anomaly_prob)
    if in_maps is None:
        return _numpy_fallback(features, anomaly_prob)
    nc = _get_compiled()
    res = run_bass_kernel_spmd(nc, in_maps, list(range(N_CORES)))
    return _combine(res.results, metas)


# revision 8
# speedup vs baseline: 1.0013x; 1.0013x over previous
"""Trainium2 Bass kernel for the contrastive loss problem.

Math (per batch element b, one NeuronCore each):
  feat (C=64, N=4000), prob (N,);  normal = prob < 0.5
  featn = l2-normalize(feat, axis=C);  s = (featn.T @ featn) / 0.1
  pos_loss = -log(mean_{m!=n, both normal} exp(s_mn) + 1e-6)
  neg_loss = mean_{m normal, n anomaly} -log(1 - sigmoid(s_mn) + 1e-6)
  result   = sum_b valid_b * (pos+neg) / max(#valid, 1)

Strategy (data-parallel over batch, 8 cores):
- pos phase: bf16 Gram upper blocks, ScalarE Exp + fused accumulate.
- neg phase: softplus(s) = s + ln(1+e^{-s}).  Sum s is linear -> exact on
  host.  The 1+e^{-s} factors are built by two engine paths feeding one
  product-fold stream:
    * ACT path (9/16 row blocks): Exp activation with scale (exact e^{-s});
    * DVE path (7/16): Schraudolph bit-trick exp - the PSUM value y =
      -(A/16) s is mapped to int16 bits (y*16 + BVAL, truncating convert on
      the DVE) which reinterpret as the bf16 value ~ e^{-s}.  BVAL is bias-
      centered so the sawtooth error cancels in expectation (validated to
      ~1e-4 of neg_sum on this input distribution).
  The neg Gram itself runs in fp8-e4m3 DoubleRow matmuls (PE does K=64 in
  one 0.5-cycle/col pass); fp8 noise on s is symmetric and sigmoid-damped.
  Folds: (1+e) via DVE tensor_scalar (4x mode), pair-products via DVE
  tensor_tensor (2x) and Pool tensor_tensor, Ln+accumulate every 8 units.
Masked (zero-padded) entries contribute exactly-known constants per path;
the host subtracts them in closed form.  Diagonal 128x128 blocks, normal
points beyond 2048 and anomaly points beyond na - na%8 are handled on the
host in f64 (cheap boundary work).
"""

import numpy as np

RW = 2048          # padded region width = 16 blocks of 128
NBLK = RW // 128   # 16 row blocks
UNIT = 2048        # PSUM staging tile width (4 banks); ping-pong 2 tiles
N_CORES = 8
EPS = 1e-6
_SQ10 = float(np.sqrt(10.0))
_A = 128.0 * 1.4426950408889634   # 128*log2(e): bf16-bits units per ln-unit
_BSHIFT = -7.3
_BVAL = 127.0 * 128.0 + _BSHIFT + 0.5   # +0.5: truncating convert -> round
_NEG_SCALE = -_A / 16.0                 # host pre-scale on rn (fp8 range)

# neg unit path assignment: True -> ACT exact-exp, False -> DVE Schraudolph
_PATH_A = [True, False, True, False, True, False, True, False,
           True, False, True, False, True, False, True, True]  # 9 A / 7 B


def _make_stream(block_col_ranges):
    segs, pos = [], 0
    for j, cs, ce in block_col_ranges:
        c = cs
        while c < ce:
            take = min(512 - (pos % 512), ce - c)
            segs.append((j, c, c + take, pos))
            pos += take
            c += take
    return segs, pos


_POS_SEGS, _POS_LEN = _make_stream([(j, 128 * j, RW) for j in range(NBLK)])
_NU_P = (_POS_LEN + UNIT - 1) // UNIT   # 9 units (last 1024 wide)
_N_GRP = (NBLK + 7) // 8                # 2 Ln groups of 8 neg units

_compiled = None


def _build():
    import concourse.bass as bass
    import concourse.mybir as mybir
    import concourse.tile as tile
    from concourse import bacc
    from concourse.hw_specs import get_activation_tables

    # Keep Exp and Ln in one activation table set (avoids table reloads).
    def _tables_pref(arch):
        t = get_activation_tables(arch)
        pref = "natural_log_exp_and_others"
        AFt = mybir.ActivationFunctionType
        return {k: (v if k == pref else v - {AFt.Exp, AFt.Ln})
                for k, v in t.items()}

    bacc.get_activation_tables = _tables_pref

    f32 = mybir.dt.float32
    bf16 = mybir.dt.bfloat16
    fp8 = mybir.dt.float8e4
    i16 = mybir.dt.int16
    AF = mybir.ActivationFunctionType
    ALU = mybir.AluOpType

    nc = bacc.Bacc("TRN2", target_bir_lowering=False, debug=False,
                   num_devices=N_CORES)
    rp_d = nc.dram_tensor("rp", [64, RW], bf16, kind="ExternalInput")
    rp8_d = nc.dram_tensor("rp8", [32, 2, RW], fp8, kind="ExternalInput")
    rn8_d = nc.dram_tensor("rn8", [32, 2, RW], fp8, kind="ExternalInput")
    accp_d = nc.dram_tensor("accp", [128, _NU_P], f32, kind="ExternalOutput")
    accn_d = nc.dram_tensor("accn", [128, _N_GRP], f32, kind="ExternalOutput")

    with tile.TileContext(nc) as tc:
        with (
            tc.tile_pool(name="sb", bufs=1) as sb,
            tc.tile_pool(name="scratch", bufs=2) as scratch_pool,
            tc.tile_pool(name="xeng", bufs=4) as xeng_pool,
            tc.tile_pool(name="psum", bufs=2, space=bass.MemorySpace.PSUM) as pp,
        ):
            rp_sb = sb.tile([64, RW], bf16, tag="rp")
            rp8_sb = sb.tile([32, 2, RW], fp8, tag="rp8")
            rn8_sb = sb.tile([32, 2, RW], fp8, tag="rn8")
            nc.sync.dma_start(out=rp_sb[:], in_=rp_d.ap())
            nc.gpsimd.dma_start(out=rp8_sb[:], in_=rp8_d.ap())
            nc.gpsimd.dma_start(out=rn8_sb[:], in_=rn8_d.ap())

            acc_p = sb.tile([128, _NU_P], f32, tag="accp")
            acc_n = sb.tile([128, _N_GRP], f32, tag="accn")

            def emit_pos_unit(u):
                base = u * UNIT
                w = min(UNIT, _POS_LEN - base)
                ptile = pp.tile([128, UNIT], f32, tag="unit")
                for (j, c0, c1, pos) in _POS_SEGS:
                    if base <= pos < base + w:
                        nc.tensor.matmul(
                            ptile[:, pos - base:pos - base + (c1 - c0)],
                            rp_sb[:, j * 128:(j + 1) * 128],
                            rp_sb[:, c0:c1],
                            start=True, stop=True,
                        )
                st = scratch_pool.tile([128, UNIT], bf16, tag="pscr")
                nc.scalar.activation(st[:, :w], ptile[:, :w], AF.Exp,
                                     accum_out=acc_p[:, u:u + 1])

            # neg fold stream state
            state = {"lt": None, "fill": 0, "grp": 0}

            def emit_neg_unit(j):
                ptile = pp.tile([128, UNIT], f32, tag="unit")
                for c in range(0, UNIT, 512):
                    nc.tensor.matmul(
                        ptile[:, c:c + 512],
                        rp8_sb[:, :, j * 128:(j + 1) * 128],
                        rn8_sb[:, :, c:c + 512],
                        start=True, stop=True,
                        perf_mode=mybir.MatmulPerfMode.DoubleRow,
                    )
                ft = scratch_pool.tile([128, UNIT], bf16, tag="ft")
                if _PATH_A[j]:
                    et = xeng_pool.tile([128, UNIT], bf16, tag="et")
                    nc.scalar.activation(et[:], ptile[:], AF.Exp,
                                         scale=float(16.0 / _A))
                    nc.vector.tensor_scalar_add(ft[:], et[:], 1.0)
                else:
                    bits = scratch_pool.tile([128, UNIT], i16, tag="bits")
                    nc.vector.tensor_scalar(
                        out=bits[:], in0=ptile[:],
                        scalar1=16.0, scalar2=float(_BVAL),
                        op0=ALU.mult, op1=ALU.add)
                    nc.vector.tensor_scalar_add(ft[:], bits[:].bitcast(bf16),
                                                1.0)
                f1 = xeng_pool.tile([128, UNIT // 2], bf16, tag="f1")
                nc.vector.tensor_tensor(f1[:], ft[:, :UNIT // 2],
                                        ft[:, UNIT // 2:], op=ALU.mult)
                f2 = scratch_pool.tile([128, UNIT // 4], bf16, tag="f2")
                nc.gpsimd.tensor_tensor(f2[:], f1[:, :UNIT // 4],
                                        f1[:, UNIT // 4:], op=ALU.mult)
                if state["lt"] is None:
                    state["lt"] = scratch_pool.tile([128, UNIT], bf16,
                                                    tag="lt", name="lt")
                    state["fill"] = 0
                lt, fill = state["lt"], state["fill"]
                nc.gpsimd.tensor_tensor(lt[:, fill:fill + UNIT // 8],
                                        f2[:, :UNIT // 8], f2[:, UNIT // 8:],
                                        op=ALU.mult)
                state["fill"] += UNIT // 8
                if state["fill"] == UNIT:
                    g = state["grp"]
                    ld = scratch_pool.tile([128, UNIT], bf16, tag="ld")
                    nc.scalar.activation(ld[:], lt[:], AF.Ln,
                                         accum_out=acc_n[:, g:g + 1])
                    state["grp"] += 1
                    state["lt"] = None

            # interleave pos and neg units so ACT (pos, path-A) and
            # DVE (path-B converts, folds) stay co-busy
            pos_left = list(range(_NU_P))
            neg_left = list(range(NBLK))
            total = _NU_P + NBLK
            pos_credit = 0.0
            for i in range(total):
                pos_credit += _NU_P / total
                if pos_left and (pos_credit >= 1.0 or not neg_left):
                    pos_credit -= 1.0
                    emit_pos_unit(pos_left.pop(0))
                else:
                    emit_neg_unit(neg_left.pop(0))

            nc.sync.dma_start(out=accp_d.ap(), in_=acc_p[:])
            nc.sync.dma_start(out=accn_d.ap(), in_=acc_n[:])

    nc.compile()
    return nc


def _get_compiled():
    global _compiled
    if _compiled is None:
        _compiled = _build()
    return _compiled


def _chain8(f):
    """Device fold-tree value of 8 equal bf16 factors: ((f^2)^2)^2 in bf16."""
    import ml_dtypes
    BF = ml_dtypes.bfloat16
    x = np.float64(f)
    for _ in range(3):
        x = np.float64(BF(x * x))
    return float(x)


def _prepare(features, anomaly_prob):
    import ml_dtypes
    BF = ml_dtypes.bfloat16
    FP8 = ml_dtypes.float8_e4m3
    feat_all = np.asarray(features, dtype=np.float32)[..., 0]
    prob_all = np.asarray(anomaly_prob, dtype=np.float32)[:, 0, :, 0]
    BS, C, N = feat_all.shape
    in_maps, metas = [], []
    for b in range(BS):
        feat, prob = feat_all[b], prob_all[b]
        normal = prob < np.float32(0.5)
        nn = int(normal.sum())
        na = N - nn
        if na > RW or nn - RW > 512:
            return None, None
        norms = np.sqrt(np.sum(feat * feat, axis=0, dtype=np.float32))
        sc = (np.float32(_SQ10) /
              np.maximum(norms, np.float32(1e-12))).astype(np.float32)
        featsc = feat * sc[None, :]
        nd = min(nn, RW)
        na_dev = na - (na % 8)
        fn_all = featsc[:, normal]
        fa_all = featsc[:, ~normal]
        rp = np.zeros((C, RW), np.float32)
        rp[:, :nd] = fn_all[:, :nd]
        rn = np.zeros((C, RW), np.float32)
        rn[:, :na_dev] = fa_all[:, :na_dev]
        rp16 = rp.astype(BF)
        rp64 = rp16.astype(np.float64)
        rn16 = fa_all.astype(BF).astype(np.float64)   # (64, na) for host math
        ov64 = fn_all[:, nd:nn].astype(BF).astype(np.float64)

        # fp8 operands for the neg phase (DoubleRow k-split 0:32 / 32:64)
        rp8 = rp16.astype(np.float32).astype(FP8)
        rn8 = (rn * np.float32(_NEG_SCALE)).astype(FP8)
        rp8_dr = np.ascontiguousarray(
            rp8.reshape(2, 32, RW).transpose(1, 0, 2))
        rn8_dr = np.ascontiguousarray(
            rn8.reshape(2, 32, RW).transpose(1, 0, 2))

        # host diag-block sums (pos): f64 on bf16 operands
        D_full = 0.0
        S2 = 0.0
        for j in range(NBLK):
            r0, r1 = 128 * j, min(128 * (j + 1), nd)
            if r0 >= r1:
                break
            blk = rp64[:, r0:r1]
            e = np.exp(blk.T @ blk)
            D_full += float(e.sum())
            S2 += float(np.trace(e))

        # pos overflow normals (beyond RW)
        pos_extra = 0.0
        if nn > nd:
            s_cross = ov64.T @ rp64[:, :nd]
            pos_extra += 2.0 * float(np.exp(s_cross).sum())
            e_oo = np.exp(ov64.T @ ov64)
            pos_extra += float(e_oo.sum()) - float(np.trace(e_oo))

        # neg host extras: overflow normals x ALL anomalies, plus device
        # normals x remainder anomalies (na_dev..na)
        neg_extra = 0.0
        if nn > nd:
            s_on = ov64.T @ rn16[:, :na]
            sig = 1.0 / (1.0 + np.exp(-s_on))
            neg_extra += float(-np.log(1.0 - sig + EPS).sum())
        if na_dev < na:
            s_rem = rp64[:, :nd].T @ rn16[:, na_dev:na]
            sig = 1.0 / (1.0 + np.exp(-s_rem))
            neg_extra += float(-np.log(1.0 - sig + EPS).sum())

        # sum of device s over the whole neg tile (masked entries are 0):
        # y = rp8.T @ rn8_scaled ; s = -(16/A) * y
        rp8_64 = rp8.astype(np.float64)
        rn8_64 = rn8.astype(np.float64)
        sum_y = float(rp8_64.sum(axis=1) @ rn8_64.sum(axis=1))
        sum_s = (-16.0 / _A) * sum_y

        # masked fold-column corrections per unit path
        bits0 = np.int16(int(_BVAL))  # y=0 -> trunc(BVAL)
        eps0 = float(np.array([bits0], np.int16).astype(np.uint16)
                     .view(BF).astype(np.float64)[0])
        fA = float(BF(2.0))
        fB = float(BF(1.0 + eps0))
        lnC_A = float(np.log(np.float64(_chain8(fA))))
        lnC_B = float(np.log(np.float64(_chain8(fB))))
        corr = 0.0
        for j in range(NBLK):
            rows_real = min(max(nd - 128 * j, 0), 128)
            masked = 128 * (UNIT // 8) - rows_real * (na_dev // 8)
            corr += masked * (lnC_A if _PATH_A[j] else lnC_B)

        metas.append((nn, na, nd, na_dev, D_full, S2, pos_extra,
                      neg_extra, sum_s, corr))
        in_maps.append({"rp": rp16, "rp8": rp8_dr, "rn8": rn8_dr})
    return in_maps, metas


def _combine(results, metas):
    per_batch, n_valid = [], 0
    for r, (nn, na, nd, na_dev, D_full, S2, pos_extra,
            neg_extra, sum_s, corr) in zip(results, metas):
        TC = float(np.asarray(r["accp"], dtype=np.float64).sum())
        TN = float(np.asarray(r["accn"], dtype=np.float64).sum())
        fake_c = 0
        for j in range(NBLK):
            rows = min(max(nd - 128 * j, 0), 128)
            cols = max(nd - 128 * j, 0)
            fake_c += 128 * (RW - 128 * j) - rows * cols
        TC_real = TC - float(fake_c)
        pos_sum = 2.0 * TC_real - D_full - S2 + pos_extra
        pos_loss = -np.log(pos_sum / max(nn * (nn - 1), 1) + EPS)
        # device neg: sum over real pairs of softplus(-s) = softplus(s) - s
        neg_sum = (TN - corr) + sum_s + neg_extra
        neg_loss = neg_sum / max(nn * na, 1)
        if nn >= 10 and na >= 5:
            n_valid += 1
            per_batch.append(pos_loss + neg_loss)
    total = np.sum(per_batch) / max(n_valid, 1) if per_batch else 0.0
    return np.asarray(total, dtype=np.float32)


def _numpy_fallback(features, anomaly_prob):
    feat_all = np.asarray(features, dtype=np.float32)[..., 0]
    prob_all = np.asarray(anomaly_prob, dtype=np.float32)[:, 0, :, 0]
    BS, C, N = feat_all.shape
    per_batch, n_valid = [], 0
    for b in range(BS):
        feat, prob = feat_all[b], prob_all[b]
        normal = prob < 0.5
        nn = int(normal.sum()); na = N - nn
        norms = np.sqrt(np.sum(feat * feat, axis=0, dtype=np.float32))
        fn = feat / np.maximum(norms, 1e-12)[None, :]
        s = (fn.T @ fn) / np.float32(0.1)
        nm, am = normal, ~normal
        eye = np.eye(N, dtype=bool)
        pm = nm[:, None] & nm[None, :] & ~eye
        pos_mean = np.where(pm, np.exp(s), 0.0).sum() / max(pm.sum(), 1)
        pos_loss = -np.log(pos_mean + EPS)
        cm = nm[:, None] & am[None, :]
        neg = np.where(cm, -np.log(1.0 - 1.0 / (1.0 + np.exp(-s)) + EPS),
                       0.0).sum() / max(cm.sum(), 1)
        if nn >= 10 and na >= 5:
            n_valid += 1
            per_batch.append(pos_loss + neg)
    total = np.sum(per_batch) / max(n_valid, 1) if per_batch else 0.0
    return np.asarray(total, dtype=np.float32)


def kernel(features, anomaly_prob):
    from concourse.bass_utils import run_bass_kernel_spmd
    in_maps, metas = _prepare(features, anomaly_prob)
    if in_maps is None:
        return _numpy_fallback(features, anomaly_prob)
    nc = _get_compiled()
    res = run_bass_kernel_spmd(nc, in_maps, list(range(N_CORES)))
    return _combine(res.results, metas)


# revision 9
# speedup vs baseline: 1.0614x; 1.0600x over previous
"""Trainium2 Bass kernel for the contrastive loss problem.

Math (per batch element b, one NeuronCore each):
  feat (C=64, N=4000), prob (N,);  normal = prob < 0.5
  featn = l2-normalize(feat, axis=C);  s = (featn.T @ featn) / 0.1
  pos_loss = -log(mean_{m!=n, both normal} exp(s_mn) + 1e-6)
  neg_loss = mean_{m normal, n anomaly} -log(1 - sigmoid(s_mn) + 1e-6)
  result   = sum_b valid_b * (pos+neg) / max(#valid, 1)

Strategy: data-parallel over batch (8 batches -> 8 cores). Host sorts points
normal-first, scales by sqrt(10) (so the Gram matrix is directly s), and
builds two zero-padded (64, 2048) bf16 operands:
  rp = first min(nn, 2048) normalized normal points
  rn = normalized anomaly points (na <= 2048 for the target inputs)
The device computes, per 128-row block j of rp:
  pos: exp-sum of rp_blk.T @ rp[:, 128j:2048]   (block upper triangle incl.
       the diagonal block; ScalarE fused accumulate out of PSUM)
  neg: softplus-sum of rp_blk.T @ rn, via Exp -> DVE product-fold (8 deep,
       bf16) -> Ln+accumulate (softplus(s) = ln(1+e^s); the |error| vs the
       reference's -log(1-sigmoid(s)+eps) is ~eps*(1+e^s), negligible).
Masked (zero-padded) entries contribute exp(0)=1 / factor 2.0 exactly; the
host subtracts them in closed form. The diagonal 128x128 blocks and any
normal points beyond 2048 are handled on the host in f64 (cheap), which is
what lets the device stream stay at 16 blocks.
"""

import numpy as np

RW = 2048          # padded region width = 16 blocks of 128
NBLK = RW // 128   # 16 row blocks
UNIT = 2048        # PSUM staging tile width (4 banks); ping-pong 2 tiles
N_CORES = 8
EPS = 1e-6
_SQ10 = float(np.sqrt(10.0))


def _make_stream(block_col_ranges):
    """Cut a concatenated (block, colrange) matmul output stream into <=512
    segments that never cross a 512-stream boundary (PSUM bank safety).
    Returns (segments, total): segments = (block, c0, c1, stream_pos)."""
    segs, pos = [], 0
    for j, cs, ce in block_col_ranges:
        c = cs
        while c < ce:
            take = min(512 - (pos % 512), ce - c)
            segs.append((j, c, c + take, pos))
            pos += take
            c += take
    return segs, pos


# pos C-stream: per block j, cols [128j, 2048) — upper blocks + diag block.
_POS_SEGS, _POS_LEN = _make_stream([(j, 128 * j, RW) for j in range(NBLK)])
_NU_P = (_POS_LEN + UNIT - 1) // UNIT   # 9 units (last 1024 wide)
_NU_N = NBLK                            # 16 neg units of exactly 2048
_N_GRP = (_NU_N + 7) // 8               # Ln groups (8 units -> 2048 cols)

_compiled = None


def _build():
    import concourse.bass as bass
    import concourse.mybir as mybir
    import concourse.tile as tile
    from concourse import bacc
    from concourse.hw_specs import get_activation_tables

    # Exp and Ln both live in the 'natural_log_exp_and_others' table set, but
    # the default placement resolves them to different sets, causing a ~1.3us
    # ACT table reload on every Exp<->Ln alternation. Steer the placement to
    # the shared set by hiding Exp/Ln from every other set. Set ORDER must be
    # preserved: act_func_set_id is the index into act_info.json's sets, and
    # NRT loads table content by that index.
    def _tables_pref(arch):
        t = get_activation_tables(arch)
        pref = "natural_log_exp_and_others"
        AFt = mybir.ActivationFunctionType
        return {k: (v if k == pref else v - {AFt.Exp, AFt.Ln})
                for k, v in t.items()}

    bacc.get_activation_tables = _tables_pref

    f32 = mybir.dt.float32
    bf16 = mybir.dt.bfloat16
    AF = mybir.ActivationFunctionType

    nc = bacc.Bacc("TRN2", target_bir_lowering=False, debug=False,
                   num_devices=N_CORES)
    rp_d = nc.dram_tensor("rp", [64, RW], bf16, kind="ExternalInput")
    rn_d = nc.dram_tensor("rn", [64, RW], bf16, kind="ExternalInput")
    accp_d = nc.dram_tensor("accp", [128, _NU_P], f32, kind="ExternalOutput")
    accn_d = nc.dram_tensor("accn", [128, _N_GRP], f32, kind="ExternalOutput")

    with tile.TileContext(nc) as tc:
        with (
            tc.tile_pool(name="sb", bufs=1) as sb,
            tc.tile_pool(name="scratch", bufs=2) as scratch_pool,
            tc.tile_pool(name="psum", bufs=2, space=bass.MemorySpace.PSUM) as pp,
        ):
            rp_sb = sb.tile([64, RW], bf16, tag="rp")
            rn_sb = sb.tile([64, RW], bf16, tag="rn")
            # separate queues so the two loads overlap
            nc.sync.dma_start(out=rp_sb[:], in_=rp_d.ap())
            nc.gpsimd.dma_start(out=rn_sb[:], in_=rn_d.ap())

            acc_p = sb.tile([128, _NU_P], f32, tag="accp")
            acc_n = sb.tile([128, _N_GRP], f32, tag="accn")

            def emit_matmuls(ptile, segs, total, u, rhs_sb):
                base = u * UNIT
                w = min(UNIT, total - base)
                for (j, c0, c1, pos) in segs:
                    if base <= pos < base + w:
                        nc.tensor.matmul(
                            ptile[:, pos - base:pos - base + (c1 - c0)],
                            rp_sb[:, j * 128:(j + 1) * 128],
                            rhs_sb[:, c0:c1],
                            start=True, stop=True,
                        )
                return w

            # pos phase (exp-sum, fused accumulate straight out of PSUM)
            for u in range(_NU_P):
                ptile = pp.tile([128, UNIT], f32, tag="unit")
                w = emit_matmuls(ptile, _POS_SEGS, _POS_LEN, u, rp_sb)
                st = scratch_pool.tile([128, UNIT], bf16, tag="scratch")
                nc.scalar.activation(st[:, :w], ptile[:, :w], AF.Exp,
                                     accum_out=acc_p[:, u:u + 1])

            # neg phase: sum softplus(s) = sum ln(1+e^s), with groups of 8
            # (1+e^s) factors folded by the (otherwise idle) DVE in bf16 so
            # the Ln pass is 8x narrower (max product (1+e^10)^8 ~ 5.7e34 is
            # inside bf16 range). ln(prod) decomposes exactly for masked
            # columns because their factor is exactly 2.0 in bf16. Folded
            # outputs of 8 units share one Ln+accumulate op.
            ltw = None
            fill = 0
            grp = 0
            for u in range(_NU_N):
                ptile = pp.tile([128, UNIT], f32, tag="unit")
                for c in range(0, UNIT, 512):
                    nc.tensor.matmul(
                        ptile[:, c:c + 512],
                        rp_sb[:, u * 128:(u + 1) * 128],
                        rn_sb[:, c:c + 512],
                        start=True, stop=True,
                    )
                w = UNIT
                h1, h2, h3 = w // 2, w // 4, w // 8
                et = scratch_pool.tile([128, UNIT], bf16, tag="scratch")
                nc.scalar.activation(et[:, :w], ptile[:, :w], AF.Exp)
                at = scratch_pool.tile([128, UNIT // 2], bf16, tag="fold_a")
                nc.vector.tensor_scalar_add(at[:, :h1], et[:, h1:w], 1.0)
                bt = scratch_pool.tile([128, UNIT // 2], bf16, tag="fold_b")
                nc.vector.scalar_tensor_tensor(
                    bt[:, :h1], et[:, :h1], 1.0, at[:, :h1],
                    op0=mybir.AluOpType.add, op1=mybir.AluOpType.mult)
                ht = scratch_pool.tile([128, UNIT // 4], bf16, tag="fold_h")
                nc.vector.tensor_tensor(
                    ht[:, :h2], bt[:, :h2], bt[:, h2:h1],
                    op=mybir.AluOpType.mult)
                if ltw is None:
                    ltw = scratch_pool.tile([128, UNIT], bf16, tag="fold_l",
                                            name="ltw")
                    fill = 0
                nc.vector.tensor_tensor(
                    ltw[:, fill:fill + h3], ht[:, :h3], ht[:, h3:h2],
                    op=mybir.AluOpType.mult)
                fill += h3
                if u % 8 == 7 or u == _NU_N - 1:
                    ld = scratch_pool.tile([128, UNIT], bf16, tag="fold_o")
                    nc.scalar.activation(ld[:, :fill], ltw[:, :fill], AF.Ln,
                                         accum_out=acc_n[:, grp:grp + 1])
                    grp += 1
                    ltw = None

            # raw accumulators out; final reduction happens on host in f64
            nc.sync.dma_start(out=accp_d.ap(), in_=acc_p[:])
            nc.sync.dma_start(out=accn_d.ap(), in_=acc_n[:])

    nc.compile()
    return nc


def _get_compiled():
    global _compiled
    if _compiled is None:
        _compiled = _build()
    return _compiled


def _prepare(features, anomaly_prob):
    """Host prep: per batch -> (rp, rn) operands + metadata for combine."""
    import ml_dtypes
    feat_all = np.asarray(features, dtype=np.float32)[..., 0]      # (8,64,4000)
    prob_all = np.asarray(anomaly_prob, dtype=np.float32)[:, 0, :, 0]
    BS, C, N = feat_all.shape
    in_maps, metas = [], []
    for b in range(BS):
        feat, prob = feat_all[b], prob_all[b]
        normal = prob < np.float32(0.5)
        nn = int(normal.sum())
        na = N - nn
        # device covers 2048 normal points and 2048 anomaly columns; the
        # host cleans up a small normal overflow. Bail out to numpy if the
        # input distribution is far from the expected ~50/50 split.
        if na > RW or nn - RW > 512:
            return None, None
        norms = np.sqrt(np.sum(feat * feat, axis=0, dtype=np.float32))
        sc = (np.float32(_SQ10) /
              np.maximum(norms, np.float32(1e-12))).astype(np.float32)
        featsc = feat * sc[None, :]
        nd = min(nn, RW)
        fn_all = featsc[:, normal]            # (64, nn) normal points
        fa_all = featsc[:, ~normal]           # (64, na)
        rp = np.zeros((C, RW), np.float32)
        rp[:, :nd] = fn_all[:, :nd]
        rn = np.zeros((C, RW), np.float32)
        rn[:, :na] = fa_all
        rp16 = rp.astype(ml_dtypes.bfloat16)
        rn16 = rn.astype(ml_dtypes.bfloat16)
        # f64 views of the bf16-rounded operands (same values the PE sees)
        rp64 = rp16.astype(np.float64)
        rn64 = rn16.astype(np.float64)
        ov64 = fn_all[:, nd:nn].astype(ml_dtypes.bfloat16).astype(np.float64)

        # host-side diagonal-block sums (f64): D_full = sum over same-block
        # normal-normal pairs (incl. m=n), S2 = sum over m=n only.
        D_full = 0.0
        S2 = 0.0
        for j in range(NBLK):
            r0, r1 = 128 * j, min(128 * (j + 1), nd)
            if r0 >= r1:
                break
            blk = rp64[:, r0:r1]
            sblk = blk.T @ blk
            e = np.exp(sblk)
            D_full += float(e.sum())
            S2 += float(np.trace(e))

        # overflow normals (beyond RW): pos pairs vs all normals + selves,
        # neg pairs vs all anomalies, in f64.
        pos_extra = 0.0
        neg_extra = 0.0
        if nn > nd:
            dev64 = rp64[:, :nd]
            s_cross = ov64.T @ dev64              # (novf, nd)
            pos_extra += 2.0 * float(np.exp(s_cross).sum())
            s_oo = ov64.T @ ov64
            e_oo = np.exp(s_oo)
            pos_extra += float(e_oo.sum()) - float(np.trace(e_oo))
            s_on = ov64.T @ rn64[:, :na]
            sig = 1.0 / (1.0 + np.exp(-s_on))
            neg_extra += float(-np.log(1.0 - sig + EPS).sum())

        metas.append((nn, na, nd, D_full, S2, pos_extra, neg_extra))
        in_maps.append({"rp": rp16, "rn": rn16})
    return in_maps, metas


def _combine(results, metas):
    LN2 = float(np.log(np.float32(2.0)))
    per_batch, n_valid = [], 0
    for r, (nn, na, nd, D_full, S2, pos_extra, neg_extra) in zip(results, metas):
        TC = float(np.asarray(r["accp"], dtype=np.float64).sum())
        TN = float(np.asarray(r["accn"], dtype=np.float64).sum())
        # pos: C-stream block j covers rows [128j,128j+128) x cols [128j, RW).
        # Real (non-padded) entries need row < nd and col < nd.
        fake_c = 0
        for j in range(NBLK):
            rows = min(max(nd - 128 * j, 0), 128)
            cols = max(nd - 128 * j, 0)
            fake_c += 128 * (RW - 128 * j) - rows * cols
        TC_real = TC - float(fake_c)          # exp(0) = 1 exactly
        pos_sum = 2.0 * TC_real - D_full - S2 + pos_extra
        pos_loss = -np.log(pos_sum / max(nn * (nn - 1), 1) + EPS)
        # neg: stream is RW rows x RW cols; real entries: row < nd, col < na
        fake_n = RW * RW - nd * na
        neg_sum = TN - fake_n * LN2 + neg_extra
        neg_loss = neg_sum / max(nn * na, 1)
        if nn >= 10 and na >= 5:
            n_valid += 1
            per_batch.append(pos_loss + neg_loss)
    total = np.sum(per_batch) / max(n_valid, 1) if per_batch else 0.0
    return np.asarray(total, dtype=np.float32)


def _numpy_fallback(features, anomaly_prob):
    feat_all = np.asarray(features, dtype=np.float32)[..., 0]
    prob_all = np.asarray(anomaly_prob, dtype=np.float32)[:, 0, :, 0]
    BS, C, N = feat_all.shape
    per_batch, n_valid = [], 0
    for b in range(BS):
        feat, prob = feat_all[b], prob_all[b]
        normal = prob < 0.5
        nn = int(normal.sum()); na = N - nn
        norms = np.sqrt(np.sum(feat * feat, axis=0, dtype=np.float32))
        fn = feat / np.maximum(norms, 1e-12)[None, :]
        s = (fn.T @ fn) / np.float32(0.1)
        nm, am = normal, ~normal
        eye = np.eye(N, dtype=bool)
        pm = nm[:, None] & nm[None, :] & ~eye
        pos_mean = np.where(pm, np.exp(s), 0.0).sum() / max(pm.sum(), 1)
        pos_loss = -np.log(pos_mean + EPS)
        cm = nm[:, None] & am[None, :]
        neg = np.where(cm, -np.log(1.0 - 1.0 / (1.0 + np.exp(-s)) + EPS),
                       0.0).sum() / max(cm.sum(), 1)
        if nn >= 10 and na >= 5:
            n_valid += 1
            per_batch.append(pos_loss + neg)
    total = np.sum(per_batch) / max(n_valid, 1) if per_batch else 0.0
    return np.asarray(total, dtype=np.float32)


def kernel(features, anomaly_prob):
    from concourse.bass_utils import run_bass_kernel_spmd
    in_maps, metas = _prepare(features, anomaly_prob)
    if in_maps is None:
        return _numpy_fallback(features, anomaly_prob)
    nc = _get_compiled()
    res = run_bass_kernel_spmd(nc, in_maps, list(range(N_CORES)))
    return _combine(res.results, metas)


# revision 12
# speedup vs baseline: 1.1545x; 1.0877x over previous
"""Trainium2 Bass kernel for the contrastive loss problem.

Math (per batch element b, one NeuronCore each):
  feat (C=64, N=4000), prob (N,);  normal = prob < 0.5
  featn = l2-normalize(feat, axis=C);  s = (featn.T @ featn) / 0.1
  pos_loss = -log(mean_{m!=n, both normal} exp(s_mn) + 1e-6)
  neg_loss = mean_{m normal, n anomaly} -log(1 - sigmoid(s_mn) + 1e-6)
  result   = sum_b valid_b * (pos+neg) / max(#valid, 1)

Strategy: data-parallel over batch (8 batches -> 8 cores). Host sorts points
normal-first, scales by sqrt(10) (so the Gram matrix is directly s), and
builds two zero-padded (64, 2048) bf16 operands:
  rp = first min(nn, 2048) normalized normal points
  rn = normalized anomaly points (na <= 2048 for the target inputs)
The device computes, per 128-row block j of rp:
  pos: exp-sum of rp_blk.T @ rp[:, 128j:2048]   (block upper triangle incl.
       the diagonal block; ScalarE fused accumulate out of PSUM)
  neg: softplus-sum of rp_blk.T @ rn, via Exp -> DVE product-fold (8 deep,
       bf16) -> Ln+accumulate (softplus(s) = ln(1+e^s); the |error| vs the
       reference's -log(1-sigmoid(s)+eps) is ~eps*(1+e^s), negligible).
Masked (zero-padded) entries contribute exp(0)=1 / factor 2.0 exactly; the
host subtracts them in closed form. The diagonal 128x128 blocks and any
normal points beyond 2048 are handled on the host in f64 (cheap), which is
what lets the device stream stay at 16 blocks.
"""

import numpy as np

RW = 2048          # padded region width = 16 blocks of 128
NBLK = RW // 128   # 16 row blocks
UNIT = 2048        # PSUM staging tile width (4 banks); ping-pong 2 tiles
N_CORES = 8
EPS = 1e-6
_SQ10 = float(np.sqrt(10.0))


def _make_stream(block_col_ranges):
    """Cut a concatenated (block, colrange) matmul output stream into <=512
    segments that never cross a 512-stream boundary (PSUM bank safety).
    Returns (segments, total): segments = (block, c0, c1, stream_pos)."""
    segs, pos = [], 0
    for j, cs, ce in block_col_ranges:
        c = cs
        while c < ce:
            take = min(512 - (pos % 512), ce - c)
            segs.append((j, c, c + take, pos))
            pos += take
            c += take
    return segs, pos


# pos U-stream: per block j, cols [128(j+1), 2048) — strict upper blocks
# only; the diagonal 128x128 blocks are recomputed on the host (D_full).
_POS_SEGS, _POS_LEN = _make_stream(
    [(j, 128 * (j + 1), RW) for j in range(NBLK - 1)])
_NU_P = (_POS_LEN + UNIT - 1) // UNIT   # 8 units (last 1024 wide)
_NU_N = NBLK                            # 16 neg units of exactly 2048
_N_GRP = (_NU_N + 7) // 8               # Ln groups (8 units -> 2048 cols)

_compiled = None


def _build():
    import concourse.bass as bass
    import concourse.mybir as mybir
    import concourse.tile as tile
    from concourse import bacc
    from concourse.hw_specs import get_activation_tables

    # Exp and Ln both live in the 'natural_log_exp_and_others' table set, but
    # the default placement resolves them to different sets, causing a ~1.3us
    # ACT table reload on every Exp<->Ln alternation. Steer the placement to
    # the shared set by hiding Exp/Ln from every other set. Set ORDER must be
    # preserved: act_func_set_id is the index into act_info.json's sets, and
    # NRT loads table content by that index.
    def _tables_pref(arch):
        t = get_activation_tables(arch)
        pref = "natural_log_exp_and_others"
        AFt = mybir.ActivationFunctionType
        return {k: (v if k == pref else v - {AFt.Exp, AFt.Ln})
                for k, v in t.items()}

    bacc.get_activation_tables = _tables_pref

    f32 = mybir.dt.float32
    bf16 = mybir.dt.bfloat16
    AF = mybir.ActivationFunctionType

    nc = bacc.Bacc("TRN2", target_bir_lowering=False, debug=False,
                   num_devices=N_CORES)
    rp_d = nc.dram_tensor("rp", [64, RW], bf16, kind="ExternalInput")
    rn_d = nc.dram_tensor("rn", [64, RW], bf16, kind="ExternalInput")
    accp_d = nc.dram_tensor("accp", [128, _NU_P], f32, kind="ExternalOutput")
    accn_d = nc.dram_tensor("accn", [128, _N_GRP], f32, kind="ExternalOutput")

    with tile.TileContext(nc) as tc:
        with (
            tc.tile_pool(name="sb", bufs=1) as sb,
            tc.tile_pool(name="scratch", bufs=2) as scratch_pool,
            tc.tile_pool(name="psum", bufs=2, space=bass.MemorySpace.PSUM) as pp,
        ):
            rp_sb = sb.tile([64, RW], bf16, tag="rp")
            rn_sb = sb.tile([64, RW], bf16, tag="rn")
            # separate queues so the two loads overlap
            nc.sync.dma_start(out=rp_sb[:], in_=rp_d.ap())
            nc.gpsimd.dma_start(out=rn_sb[:], in_=rn_d.ap())

            acc_p = sb.tile([128, _NU_P], f32, tag="accp")
            acc_n = sb.tile([128, _N_GRP], f32, tag="accn")

            def emit_matmuls(ptile, segs, total, u, rhs_sb):
                base = u * UNIT
                w = min(UNIT, total - base)
                for (j, c0, c1, pos) in segs:
                    if base <= pos < base + w:
                        nc.tensor.matmul(
                            ptile[:, pos - base:pos - base + (c1 - c0)],
                            rp_sb[:, j * 128:(j + 1) * 128],
                            rhs_sb[:, c0:c1],
                            start=True, stop=True,
                        )
                return w

            # Ln flushes are deferred by one unit: the Ln for a full fold
            # group is emitted right AFTER the next unit's Exp, so the DVE
            # fold drain overlaps useful ACT work instead of stalling it.
            # The last group's Ln lands inside the pos phase.
            state = {"ltw": None, "fill": 0, "grp": 0, "pending": None}

            def flush_pending_ln():
                if state["pending"] is not None:
                    ltw, g, w = state["pending"]
                    ld = scratch_pool.tile([128, UNIT], bf16, tag="fold_o",
                                           name="ld")
                    nc.scalar.activation(ld[:, :w], ltw[:, :w], AF.Ln,
                                         accum_out=acc_n[:, g:g + 1])
                    state["pending"] = None

            # neg phase first: sum softplus(s) = sum ln(1+e^s), with groups
            # of 8 (1+e^s) factors folded by the DVE in bf16 so the Ln pass
            # is 8x narrower (max product (1+e^10)^8 ~ 5.7e34 is inside bf16
            # range). ln(prod) decomposes exactly for masked columns because
            # their factor is exactly 2.0 in bf16.
            for u in range(_NU_N):
                ptile = pp.tile([128, UNIT], f32, tag="unit")
                for c in range(0, UNIT, 512):
                    nc.tensor.matmul(
                        ptile[:, c:c + 512],
                        rp_sb[:, u * 128:(u + 1) * 128],
                        rn_sb[:, c:c + 512],
                        start=True, stop=True,
                    )
                w = UNIT
                h1, h2, h3 = w // 2, w // 4, w // 8
                et = scratch_pool.tile([128, UNIT], bf16, tag="scratch")
                nc.scalar.activation(et[:, :w], ptile[:, :w], AF.Exp)
                flush_pending_ln()
                ft = scratch_pool.tile([128, UNIT], bf16, tag="fold_f")
                nc.vector.tensor_scalar_add(ft[:, :w], et[:, :w], 1.0)
                ht = scratch_pool.tile([128, UNIT // 2], bf16, tag="fold_h")
                nc.vector.tensor_tensor(
                    ht[:, :h1], ft[:, :h1], ft[:, h1:w],
                    op=mybir.AluOpType.mult)
                gt = scratch_pool.tile([128, UNIT // 4], bf16, tag="fold_g")
                nc.vector.tensor_tensor(
                    gt[:, :h2], ht[:, :h2], ht[:, h2:h1],
                    op=mybir.AluOpType.mult)
                if state["ltw"] is None:
                    state["ltw"] = scratch_pool.tile(
                        [128, UNIT], bf16, tag="fold_l", name="ltw")
                    state["fill"] = 0
                ltw, fill = state["ltw"], state["fill"]
                nc.vector.tensor_tensor(
                    ltw[:, fill:fill + h3], gt[:, :h3], gt[:, h3:h2],
                    op=mybir.AluOpType.mult)
                state["fill"] += h3
                if state["fill"] == UNIT or u == _NU_N - 1:
                    state["pending"] = (ltw, state["grp"], state["fill"])
                    state["grp"] += 1
                    state["ltw"] = None

            # pos phase (exp-sum, fused accumulate straight out of PSUM)
            for u in range(_NU_P):
                ptile = pp.tile([128, UNIT], f32, tag="unit")
                w = emit_matmuls(ptile, _POS_SEGS, _POS_LEN, u, rp_sb)
                st = scratch_pool.tile([128, UNIT], bf16, tag="scratch")
                nc.scalar.activation(st[:, :w], ptile[:, :w], AF.Exp,
                                     accum_out=acc_p[:, u:u + 1])
                flush_pending_ln()
            flush_pending_ln()

            # raw accumulators out; final reduction happens on host in f64
            nc.sync.dma_start(out=accp_d.ap(), in_=acc_p[:])
            nc.sync.dma_start(out=accn_d.ap(), in_=acc_n[:])

    nc.compile()
    return nc


def _get_compiled():
    global _compiled
    if _compiled is None:
        _compiled = _build()
    return _compiled


def _prepare(features, anomaly_prob):
    """Host prep: per batch -> (rp, rn) operands + metadata for combine."""
    import ml_dtypes
    feat_all = np.asarray(features, dtype=np.float32)[..., 0]      # (8,64,4000)
    prob_all = np.asarray(anomaly_prob, dtype=np.float32)[:, 0, :, 0]
    BS, C, N = feat_all.shape
    in_maps, metas = [], []
    for b in range(BS):
        feat, prob = feat_all[b], prob_all[b]
        normal = prob < np.float32(0.5)
        nn = int(normal.sum())
        na = N - nn
        # device covers 2048 normal points and 2048 anomaly columns; the
        # host cleans up a small normal overflow. Bail out to numpy if the
        # input distribution is far from the expected ~50/50 split.
        if na > RW or nn - RW > 512:
            return None, None
        norms = np.sqrt(np.sum(feat * feat, axis=0, dtype=np.float32))
        sc = (np.float32(_SQ10) /
              np.maximum(norms, np.float32(1e-12))).astype(np.float32)
        featsc = feat * sc[None, :]
        nd = min(nn, RW)
        fn_all = featsc[:, normal]            # (64, nn) normal points
        fa_all = featsc[:, ~normal]           # (64, na)
        rp = np.zeros((C, RW), np.float32)
        rp[:, :nd] = fn_all[:, :nd]
        rn = np.zeros((C, RW), np.float32)
        rn[:, :na] = fa_all
        rp16 = rp.astype(ml_dtypes.bfloat16)
        rn16 = rn.astype(ml_dtypes.bfloat16)
        # f64 views of the bf16-rounded operands (same values the PE sees)
        rp64 = rp16.astype(np.float64)
        rn64 = rn16.astype(np.float64)
        ov64 = fn_all[:, nd:nn].astype(ml_dtypes.bfloat16).astype(np.float64)

        # host-side diagonal-block sums (f64): D_full = sum over same-block
        # normal-normal pairs (incl. m=n), S2 = sum over m=n only.
        D_full = 0.0
        S2 = 0.0
        for j in range(NBLK):
            r0, r1 = 128 * j, min(128 * (j + 1), nd)
            if r0 >= r1:
                break
            blk = rp64[:, r0:r1]
            sblk = blk.T @ blk
            e = np.exp(sblk)
            D_full += float(e.sum())
            S2 += float(np.trace(e))

        # overflow normals (beyond RW): pos pairs vs all normals + selves,
        # neg pairs vs all anomalies, in f64.
        pos_extra = 0.0
        neg_extra = 0.0
        if nn > nd:
            dev64 = rp64[:, :nd]
            s_cross = ov64.T @ dev64              # (novf, nd)
            pos_extra += 2.0 * float(np.exp(s_cross).sum())
            s_oo = ov64.T @ ov64
            e_oo = np.exp(s_oo)
            pos_extra += float(e_oo.sum()) - float(np.trace(e_oo))
            s_on = ov64.T @ rn64[:, :na]
            sig = 1.0 / (1.0 + np.exp(-s_on))
            neg_extra += float(-np.log(1.0 - sig + EPS).sum())

        metas.append((nn, na, nd, D_full, S2, pos_extra, neg_extra))
        in_maps.append({"rp": rp16, "rn": rn16})
    return in_maps, metas


def _combine(results, metas):
    LN2 = float(np.log(np.float32(2.0)))
    per_batch, n_valid = [], 0
    for r, (nn, na, nd, D_full, S2, pos_extra, neg_extra) in zip(results, metas):
        TC = float(np.asarray(r["accp"], dtype=np.float64).sum())
        TN = float(np.asarray(r["accn"], dtype=np.float64).sum())
        # pos: U-stream block j covers rows [128j,128j+128) x cols
        # [128(j+1), RW). Real (non-padded) entries need row < nd, col < nd.
        fake_c = 0
        for j in range(NBLK - 1):
            rows = min(max(nd - 128 * j, 0), 128)
            cols = max(nd - 128 * (j + 1), 0)
            fake_c += 128 * (RW - 128 * (j + 1)) - rows * cols
        TU_real = TC - float(fake_c)          # exp(0) = 1 exactly
        pos_sum = 2.0 * TU_real + (D_full - S2) + pos_extra
        pos_loss = -np.log(pos_sum / max(nn * (nn - 1), 1) + EPS)
        # neg: stream is RW rows x RW cols; real entries: row < nd, col < na
        fake_n = RW * RW - nd * na
        neg_sum = TN - fake_n * LN2 + neg_extra
        neg_loss = neg_sum / max(nn * na, 1)
        if nn >= 10 and na >= 5:
            n_valid += 1
            per_batch.append(pos_loss + neg_loss)
    total = np.sum(per_batch) / max(n_valid, 1) if per_batch else 0.0
    return np.asarray(total, dtype=np.float32)


def _numpy_fallback(features, anomaly_prob):
    feat_all = np.asarray(features, dtype=np.float32)[..., 0]
    prob_all = np.asarray(anomaly_prob, dtype=np.float32)[:, 0, :, 0]
    BS, C, N = feat_all.shape
    per_batch, n_valid = [], 0
    for b in range(BS):
        feat, prob = feat_all[b], prob_all[b]
        normal = prob < 0.5
        nn = int(normal.sum()); na = N - nn
        norms = np.sqrt(np.sum(feat * feat, axis=0, dtype=np.float32))
        fn = feat / np.maximum(norms, 1e-12)[None, :]
        s = (fn.T @ fn) / np.float32(0.1)
        nm, am = normal, ~normal
        eye = np.eye(N, dtype=bool)
        pm = nm[:, None] & nm[None, :] & ~eye
        pos_mean = np.where(pm, np.exp(s), 0.0).sum() / max(pm.sum(), 1)
        pos_loss = -np.log(pos_mean + EPS)
        cm = nm[:, None] & am[None, :]
        neg = np.where(cm, -np.log(1.0 - 1.0 / (1.0 + np.exp(-s)) + EPS),
                       0.0).sum() / max(cm.sum(), 1)
        if nn >= 10 and na >= 5:
            n_valid += 1
            per_batch.append(pos_loss + neg)
    total = np.sum(per_batch) / max(n_valid, 1) if per_batch else 0.0
    return np.asarray(total, dtype=np.float32)


def kernel(features, anomaly_prob):
    from concourse.bass_utils import run_bass_kernel_spmd
    in_maps, metas = _prepare(features, anomaly_prob)
    if in_maps is None:
        return _numpy_fallback(features, anomaly_prob)
    nc = _get_compiled()
    res = run_bass_kernel_spmd(nc, in_maps, list(range(N_CORES)))
    return _combine(res.results, metas)


# revision 16
# speedup vs baseline: 1.2058x; 1.0444x over previous
"""Trainium2 Bass kernel for the contrastive loss problem.

Math (per batch element b, one NeuronCore each):
  feat (C=64, N=4000), prob (N,);  normal = prob < 0.5
  featn = l2-normalize(feat, axis=C);  s = (featn.T @ featn) / 0.1
  pos_loss = -log(mean_{m!=n, both normal} exp(s_mn) + 1e-6)
  neg_loss = mean_{m normal, n anomaly} -log(1 - sigmoid(s_mn) + 1e-6)
  result   = sum_b valid_b * (pos+neg) / max(#valid, 1)

Strategy: data-parallel over batch (8 batches -> 8 cores). Host sorts points
normal-first, scales by sqrt(10) (so the Gram matrix is directly s), and
builds two zero-padded (64, 2048) bf16 operands:
  rp = first min(nn, 2048) normalized normal points
  rn = normalized anomaly points (na <= 2048 for the target inputs)
The device computes, per 128-row block j of rp:
  pos: exp-sum of rp_blk.T @ rp[:, 128j:2048]   (block upper triangle incl.
       the diagonal block; ScalarE fused accumulate out of PSUM)
  neg: softplus-sum of rp_blk.T @ rn, via Exp -> DVE product-fold (8 deep,
       bf16) -> Ln+accumulate (softplus(s) = ln(1+e^s); the |error| vs the
       reference's -log(1-sigmoid(s)+eps) is ~eps*(1+e^s), negligible).
Masked (zero-padded) entries contribute exp(0)=1 / factor 2.0 exactly; the
host subtracts them in closed form. The diagonal 128x128 blocks and any
normal points beyond 2048 are handled on the host in f64 (cheap), which is
what lets the device stream stay at 16 blocks.
"""

import numpy as np

RW = 2048          # padded region width = 16 blocks of 128
NBLK = RW // 128   # 16 row blocks
UNIT = 2048        # PSUM staging tile width (4 banks); ping-pong 2 tiles
N_CORES = 8
EPS = 1e-6
_SQ10 = float(np.sqrt(10.0))


def _make_stream(block_col_ranges):
    """Cut a concatenated (block, colrange) matmul output stream into <=512
    segments that never cross a 512-stream boundary (PSUM bank safety).
    Returns (segments, total): segments = (block, c0, c1, stream_pos)."""
    segs, pos = [], 0
    for j, cs, ce in block_col_ranges:
        c = cs
        while c < ce:
            take = min(512 - (pos % 512), ce - c)
            segs.append((j, c, c + take, pos))
            pos += take
            c += take
    return segs, pos


# pos U-stream: per block j, cols [128(j+1), 2048) — strict upper blocks
# only; the diagonal 128x128 blocks are recomputed on the host (D_full).
_POS_SEGS, _POS_LEN = _make_stream(
    [(j, 128 * (j + 1), RW) for j in range(NBLK - 1)])
_NU_P = (_POS_LEN + UNIT - 1) // UNIT   # 8 units (last 1024 wide)
_NU_N = NBLK                            # 16 neg units of exactly 2048
_N_GRP = (_NU_N + 15) // 16             # Ln groups (16 units -> 2048 cols)

_compiled = None


def _build():
    import concourse.bass as bass
    import concourse.mybir as mybir
    import concourse.tile as tile
    from concourse import bacc
    from concourse.hw_specs import get_activation_tables

    # Exp and Ln both live in the 'natural_log_exp_and_others' table set, but
    # the default placement resolves them to different sets, causing a ~1.3us
    # ACT table reload on every Exp<->Ln alternation. Steer the placement to
    # the shared set by hiding Exp/Ln from every other set. Set ORDER must be
    # preserved: act_func_set_id is the index into act_info.json's sets, and
    # NRT loads table content by that index.
    def _tables_pref(arch):
        t = get_activation_tables(arch)
        pref = "natural_log_exp_and_others"
        AFt = mybir.ActivationFunctionType
        return {k: (v if k == pref else v - {AFt.Exp, AFt.Ln})
                for k, v in t.items()}

    bacc.get_activation_tables = _tables_pref

    f32 = mybir.dt.float32
    bf16 = mybir.dt.bfloat16
    AF = mybir.ActivationFunctionType

    nc = bacc.Bacc("TRN2", target_bir_lowering=False, debug=False,
                   num_devices=N_CORES)
    rp_d = nc.dram_tensor("rp", [64, RW], bf16, kind="ExternalInput")
    rn_d = nc.dram_tensor("rn", [64, RW], bf16, kind="ExternalInput")
    accp_d = nc.dram_tensor("accp", [128, _NU_P], f32, kind="ExternalOutput")
    accn_d = nc.dram_tensor("accn", [128, _N_GRP], f32, kind="ExternalOutput")

    with tile.TileContext(nc) as tc:
        with (
            tc.tile_pool(name="sb", bufs=1) as sb,
            tc.tile_pool(name="scratch", bufs=2) as scratch_pool,
            tc.tile_pool(name="psum", bufs=2, space=bass.MemorySpace.PSUM) as pp,
        ):
            rp_sb = sb.tile([64, RW], bf16, tag="rp")
            rn_sb = sb.tile([64, RW], bf16, tag="rn")
            # chunked loads on two queues: the first neg unit's matmul
            # segments only need rn[:, c:c+512] and rp[:, 0:128], so the PE
            # can start ~1us earlier than with monolithic loads.
            for c in range(0, RW, 512):
                nc.sync.dma_start(out=rn_sb[:, c:c + 512],
                                  in_=rn_d.ap()[:, c:c + 512])
            nc.gpsimd.dma_start(out=rp_sb[:, 0:128], in_=rp_d.ap()[:, 0:128])
            nc.gpsimd.dma_start(out=rp_sb[:, 128:RW],
                                in_=rp_d.ap()[:, 128:RW])

            acc_p = sb.tile([128, _NU_P], f32, tag="accp")
            acc_n = sb.tile([128, _N_GRP], f32, tag="accn")

            def emit_matmuls(ptile, segs, total, u, rhs_sb):
                base = u * UNIT
                w = min(UNIT, total - base)
                for (j, c0, c1, pos) in segs:
                    if base <= pos < base + w:
                        nc.tensor.matmul(
                            ptile[:, pos - base:pos - base + (c1 - c0)],
                            rp_sb[:, j * 128:(j + 1) * 128],
                            rhs_sb[:, c0:c1],
                            start=True, stop=True,
                        )
                return w

            # Ln flushes are deferred by one unit: the Ln for a full fold
            # group is emitted right AFTER the next unit's Exp, so the DVE
            # fold drain overlaps useful ACT work instead of stalling it.
            # The last group's Ln lands inside the pos phase.
            state = {"ltw": None, "fill": 0, "grp": 0, "pending": None}

            def flush_pending_ln():
                if state["pending"] is not None:
                    ltw, g, w = state["pending"]
                    ld = scratch_pool.tile([128, UNIT], bf16, tag="fold_o",
                                           name="ld")
                    nc.scalar.activation(ld[:, :w], ltw[:, :w], AF.Ln,
                                         accum_out=acc_n[:, g:g + 1])
                    state["pending"] = None

            # neg phase first: sum softplus(s) = sum ln(1+e^s), with groups
            # of 8 (1+e^s) factors folded by the DVE in bf16 so the Ln pass
            # is 8x narrower (max product (1+e^10)^8 ~ 5.7e34 is inside bf16
            # range). ln(prod) decomposes exactly for masked columns because
            # their factor is exactly 2.0 in bf16.
            for u in range(_NU_N):
                ptile = pp.tile([128, UNIT], f32, tag="unit")
                for c in range(0, UNIT, 512):
                    nc.tensor.matmul(
                        ptile[:, c:c + 512],
                        rp_sb[:, u * 128:(u + 1) * 128],
                        rn_sb[:, c:c + 512],
                        start=True, stop=True,
                    )
                w = UNIT
                h1, h2, h3, h4 = w // 2, w // 4, w // 8, w // 16
                et = scratch_pool.tile([128, UNIT], bf16, tag="scratch")
                nc.scalar.activation(et[:, :w], ptile[:, :w], AF.Exp)
                flush_pending_ln()
                # factors are (1+e^s)/4 so a 16-deep fold stays inside bf16
                # range (max (1+e^5.9)^16/4^16 ~ e^72); masked factors become
                # exactly 0.5 (exponent shift, no rounding).
                ft = scratch_pool.tile([128, UNIT], bf16, tag="fold_f")
                nc.vector.tensor_scalar(
                    out=ft[:, :w], in0=et[:, :w], scalar1=0.25, scalar2=0.25,
                    op0=mybir.AluOpType.mult, op1=mybir.AluOpType.add)
                ht = scratch_pool.tile([128, UNIT // 2], bf16, tag="fold_h")
                nc.vector.tensor_tensor(
                    ht[:, :h1], ft[:, :h1], ft[:, h1:w],
                    op=mybir.AluOpType.mult)
                gt = scratch_pool.tile([128, UNIT // 4], bf16, tag="fold_g")
                nc.vector.tensor_tensor(
                    gt[:, :h2], ht[:, :h2], ht[:, h2:h1],
                    op=mybir.AluOpType.mult)
                kt = scratch_pool.tile([128, UNIT // 8], bf16, tag="fold_k")
                nc.vector.tensor_tensor(
                    kt[:, :h3], gt[:, :h3], gt[:, h3:h2],
                    op=mybir.AluOpType.mult)
                if state["ltw"] is None:
                    state["ltw"] = scratch_pool.tile(
                        [128, UNIT], bf16, tag="fold_l", name="ltw")
                    state["fill"] = 0
                ltw, fill = state["ltw"], state["fill"]
                nc.vector.tensor_tensor(
                    ltw[:, fill:fill + h4], kt[:, :h4], kt[:, h4:h3],
                    op=mybir.AluOpType.mult)
                state["fill"] += h4
                if state["fill"] == UNIT or u == _NU_N - 1:
                    state["pending"] = (ltw, state["grp"], state["fill"])
                    state["grp"] += 1
                    state["ltw"] = None

            # pos phase (exp-sum, fused accumulate straight out of PSUM)
            for u in range(_NU_P):
                ptile = pp.tile([128, UNIT], f32, tag="unit")
                w = emit_matmuls(ptile, _POS_SEGS, _POS_LEN, u, rp_sb)
                st = scratch_pool.tile([128, UNIT], bf16, tag="scratch")
                nc.scalar.activation(st[:, :w], ptile[:, :w], AF.Exp,
                                     accum_out=acc_p[:, u:u + 1])
                flush_pending_ln()
            flush_pending_ln()

            # raw accumulators out; final reduction happens on host in f64
            nc.sync.dma_start(out=accp_d.ap(), in_=acc_p[:])
            nc.sync.dma_start(out=accn_d.ap(), in_=acc_n[:])

    nc.compile()
    return nc


def _get_compiled():
    global _compiled
    if _compiled is None:
        _compiled = _build()
    return _compiled


def _prepare(features, anomaly_prob):
    """Host prep: per batch -> (rp, rn) operands + metadata for combine."""
    import ml_dtypes
    feat_all = np.asarray(features, dtype=np.float32)[..., 0]      # (8,64,4000)
    prob_all = np.asarray(anomaly_prob, dtype=np.float32)[:, 0, :, 0]
    BS, C, N = feat_all.shape
    in_maps, metas = [], []
    for b in range(BS):
        feat, prob = feat_all[b], prob_all[b]
        normal = prob < np.float32(0.5)
        nn = int(normal.sum())
        na = N - nn
        # device covers 2048 normal points and 2048 anomaly columns; the
        # host cleans up a small normal overflow. Bail out to numpy if the
        # input distribution is far from the expected ~50/50 split.
        if na > RW or nn - RW > 512:
            return None, None
        norms = np.sqrt(np.sum(feat * feat, axis=0, dtype=np.float32))
        sc = (np.float32(_SQ10) /
              np.maximum(norms, np.float32(1e-12))).astype(np.float32)
        featsc = feat * sc[None, :]
        nd = min(nn, RW)
        fn_all = featsc[:, normal]            # (64, nn) normal points
        fa_all = featsc[:, ~normal]           # (64, na)
        rp = np.zeros((C, RW), np.float32)
        rp[:, :nd] = fn_all[:, :nd]
        rn = np.zeros((C, RW), np.float32)
        rn[:, :na] = fa_all
        rp16 = rp.astype(ml_dtypes.bfloat16)
        rn16 = rn.astype(ml_dtypes.bfloat16)
        # f64 views of the bf16-rounded operands (same values the PE sees)
        rp64 = rp16.astype(np.float64)
        rn64 = rn16.astype(np.float64)
        ov64 = fn_all[:, nd:nn].astype(ml_dtypes.bfloat16).astype(np.float64)

        # host-side diagonal-block sums (f64): D_full = sum over same-block
        # normal-normal pairs (incl. m=n), S2 = sum over m=n only.
        D_full = 0.0
        S2 = 0.0
        for j in range(NBLK):
            r0, r1 = 128 * j, min(128 * (j + 1), nd)
            if r0 >= r1:
                break
            blk = rp64[:, r0:r1]
            sblk = blk.T @ blk
            e = np.exp(sblk)
            D_full += float(e.sum())
            S2 += float(np.trace(e))

        # overflow normals (beyond RW): pos pairs vs all normals + selves,
        # neg pairs vs all anomalies, in f64.
        pos_extra = 0.0
        neg_extra = 0.0
        if nn > nd:
            dev64 = rp64[:, :nd]
            s_cross = ov64.T @ dev64              # (novf, nd)
            pos_extra += 2.0 * float(np.exp(s_cross).sum())
            s_oo = ov64.T @ ov64
            e_oo = np.exp(s_oo)
            pos_extra += float(e_oo.sum()) - float(np.trace(e_oo))
            s_on = ov64.T @ rn64[:, :na]
            sig = 1.0 / (1.0 + np.exp(-s_on))
            neg_extra += float(-np.log(1.0 - sig + EPS).sum())

        metas.append((nn, na, nd, D_full, S2, pos_extra, neg_extra))
        in_maps.append({"rp": rp16, "rn": rn16})
    return in_maps, metas


def _combine(results, metas):
    LN2 = float(np.log(np.float32(2.0)))
    per_batch, n_valid = [], 0
    for r, (nn, na, nd, D_full, S2, pos_extra, neg_extra) in zip(results, metas):
        TC = float(np.asarray(r["accp"], dtype=np.float64).sum())
        TN = float(np.asarray(r["accn"], dtype=np.float64).sum())
        # pos: U-stream block j covers rows [128j,128j+128) x cols
        # [128(j+1), RW). Real (non-padded) entries need row < nd, col < nd.
        fake_c = 0
        for j in range(NBLK - 1):
            rows = min(max(nd - 128 * j, 0), 128)
            cols = max(nd - 128 * (j + 1), 0)
            fake_c += 128 * (RW - 128 * (j + 1)) - rows * cols
        TU_real = TC - float(fake_c)          # exp(0) = 1 exactly
        pos_sum = 2.0 * TU_real + (D_full - S2) + pos_extra
        pos_loss = -np.log(pos_sum / max(nn * (nn - 1), 1) + EPS)
        # neg: stream is RW rows x RW cols; real entries (row < nd, col < na)
        # contribute softplus(s) - 2*ln2 each (the /4 factor scaling),
        # masked entries exactly -ln2 each.
        neg_sum = TN + LN2 * (RW * RW + nd * na) + neg_extra
        neg_loss = neg_sum / max(nn * na, 1)
        if nn >= 10 and na >= 5:
            n_valid += 1
            per_batch.append(pos_loss + neg_loss)
    total = np.sum(per_batch) / max(n_valid, 1) if per_batch else 0.0
    return np.asarray(total, dtype=np.float32)


def _numpy_fallback(features, anomaly_prob):
    feat_all = np.asarray(features, dtype=np.float32)[..., 0]
    prob_all = np.asarray(anomaly_prob, dtype=np.float32)[:, 0, :, 0]
    BS, C, N = feat_all.shape
    per_batch, n_valid = [], 0
    for b in range(BS):
        feat, prob = feat_all[b], prob_all[b]
        normal = prob < 0.5
        nn = int(normal.sum()); na = N - nn
        norms = np.sqrt(np.sum(feat * feat, axis=0, dtype=np.float32))
        fn = feat / np.maximum(norms, 1e-12)[None, :]
        s = (fn.T @ fn) / np.float32(0.1)
        nm, am = normal, ~normal
        eye = np.eye(N, dtype=bool)
        pm = nm[:, None] & nm[None, :] & ~eye
        pos_mean = np.where(pm, np.exp(s), 0.0).sum() / max(pm.sum(), 1)
        pos_loss = -np.log(pos_mean + EPS)
        cm = nm[:, None] & am[None, :]
        neg = np.where(cm, -np.log(1.0 - 1.0 / (1.0 + np.exp(-s)) + EPS),
                       0.0).sum() / max(cm.sum(), 1)
        if nn >= 10 and na >= 5:
            n_valid += 1
            per_batch.append(pos_loss + neg)
    total = np.sum(per_batch) / max(n_valid, 1) if per_batch else 0.0
    return np.asarray(total, dtype=np.float32)


def kernel(features, anomaly_prob):
    from concourse.bass_utils import run_bass_kernel_spmd
    in_maps, metas = _prepare(features, anomaly_prob)
    if in_maps is None:
        return _numpy_fallback(features, anomaly_prob)
    nc = _get_compiled()
    res = run_bass_kernel_spmd(nc, in_maps, list(range(N_CORES)))
    return _combine(res.results, metas)
